# revision 10
# baseline (speedup 1.0000x reference)
# Dynamic convolution (CondConv-style) Trainium2 Bass kernel.
#
# Problem: x [16, 128, 128, 128]; per-sample attention over K=4 expert
# 3x3 conv kernels; per-sample aggregated conv + bias.
#
# Strategy: data-parallel over batch, 2 samples per core on 8 cores.
# Per core / per sample:
#   1. DMA x into SBUF as [C=128, 130, 130] with a zero halo (pad=1).
#   2. DVE reduce_sum over free dims -> pooled mean [C, 1].
#   3. Attention: two tiny matmuls + softmax over K=4 (free dim).
#   4. Expert mixing fused with transpose: for each of 9 taps,
#      agg_wT[ci, co] = sum_k att_k * W_k[co, ci, tap] via 4 accumulating
#      PE transpose-matmuls whose moving operand is att_k * I.
#   5. Conv: for each 4-row output chunk (512 cols), 9 PSUM-accumulated
#      matmuls rhs = shifted window of the padded x.
#   6. Drain: bias add (per-partition scalar) PSUM->SBUF, DMA to DRAM.
import os

import numpy as np

B, C, H, W = 16, 128, 128, 128
K, HID, KS = 4, 64, 3
TEMP = 30.0
N_CORES = 8
BPC = B // N_CORES  # samples per core
HP, WP = H + 2, W + 2  # padded spatial
ROWS_PER_CHUNK = 4
NCHUNK = H // ROWS_PER_CHUNK
TAPS = KS * KS

_cache = {}


def _build(conv_f32r: bool):
    """Build + compile the Bass program (same program for all 8 cores)."""
    import concourse.bacc as bacc
    import concourse.mybir as mybir
    import concourse.tile as tile
    from concourse.masks import make_identity

    fp32 = mybir.dt.float32
    f32r = mybir.dt.float32r
    AF = mybir.ActivationFunctionType
    AX = mybir.AxisListType

    nc = bacc.Bacc(
        "TRN2",
        target_bir_lowering=False,
        debug=False,
        enable_asserts=False,
        num_devices=N_CORES,
    )

    x_d = nc.dram_tensor("x", (BPC, C, H, W), fp32, kind="ExternalInput").ap()
    w1_d = nc.dram_tensor("att_w1", (HID, C), fp32, kind="ExternalInput").ap()
    w2_d = nc.dram_tensor("att_w2", (K, HID), fp32, kind="ExternalInput").ap()
    wgt_d = nc.dram_tensor("weight", (K, C, C, KS, KS), fp32, kind="ExternalInput").ap()
    bias_d = nc.dram_tensor("bias", (K, C), fp32, kind="ExternalInput").ap()
    out_d = nc.dram_tensor("out", (BPC, C, H, W), fp32, kind="ExternalOutput").ap()

    wgt_flat = wgt_d.rearrange("k o i kh kw -> k o (i kh kw)")
    out_flat = out_d.rearrange("b c h w -> b c (h w)")

    # fp32r matmul operands must be written by a compute op that rounds to
    # fp32r; DMA alone does not qualify. So x goes HBM->SBUF contiguous,
    # then a DVE copy re-lays it into the padded tile with fp32r output.
    conv_dt = f32r if conv_f32r else fp32

    from contextlib import ExitStack

    with tile.TileContext(nc) as tc, ExitStack() as ctx:
        consts = ctx.enter_context(tc.tile_pool(name="consts", bufs=1))
        xpool = ctx.enter_context(tc.tile_pool(name="xpool", bufs=1))
        wpool = ctx.enter_context(tc.tile_pool(name="wpool", bufs=1))
        smalls = ctx.enter_context(tc.tile_pool(name="smalls", bufs=1))
        stage = ctx.enter_context(tc.tile_pool(name="stage", bufs=6))
        cpsum = ctx.enter_context(tc.tile_pool(name="cpsum", bufs=5, space="PSUM"))
        apsum = ctx.enter_context(tc.tile_pool(name="apsum", bufs=2, space="PSUM"))
        spsum = ctx.enter_context(tc.tile_pool(name="spsum", bufs=1, space="PSUM"))

        # ---- global constants ----
        ident = consts.tile([C, C], fp32, name="ident")
        make_identity(nc, ident)
        ones_row = consts.tile([1, C], fp32, name="ones_row")
        nc.vector.memset(ones_row, 1.0)
        zero_col = consts.tile([C, HP], fp32, name="zero_col")
        nc.vector.memset(zero_col, 0.0)

        w1T = consts.tile([C, HID], fp32, name="w1T")
        nc.sync.dma_start(out=w1T, in_=w1_d.rearrange("h c -> c h"))
        w2T = consts.tile([HID, K], fp32, name="w2T")
        nc.sync.dma_start(out=w2T, in_=w2_d.rearrange("k h -> h k"))
        bias_sb = consts.tile([K, C], fp32, name="bias_sb")
        nc.sync.dma_start(out=bias_sb, in_=bias_d)

        # bias transposed to [C(out), K] via PE transpose
        biasT_ps = spsum.tile([C, K], fp32, name="biasT_ps", tag="sps")
        nc.tensor.matmul(
            biasT_ps, bias_sb, ident[:K, :K], is_transpose=True, start=True, stop=True
        )
        biasT = consts.tile([C, K], fp32, name="biasT")
        nc.vector.tensor_copy(biasT, biasT_ps)

        # expert weight bank, [co, ci, tap] per expert (contiguous DMA)
        w_sb = []
        for k in range(K):
            wk = wpool.tile([C, C, TAPS], fp32, name=f"w_sb{k}")
            nc.sync.dma_start(out=wk, in_=wgt_flat[k].rearrange("o (i t) -> o i t", t=TAPS))
            w_sb.append(wk)

        xtmp_pool = ctx.enter_context(tc.tile_pool(name="xtmp", bufs=3))
        QROWS = 16  # x staging chunk height
        for b in range(BPC):
            # ---- load x (contiguous DMA), re-lay into padded tile ----
            x_pad = xpool.tile([C, HP, WP], conv_dt, name=f"x_pad{b}")
            nc.vector.tensor_copy(x_pad[:, 0, :], zero_col)
            nc.vector.tensor_copy(x_pad[:, HP - 1, :], zero_col)
            nc.vector.tensor_copy(x_pad[:, :, 0], zero_col)
            nc.vector.tensor_copy(x_pad[:, :, WP - 1], zero_col)
            for q in range(H // QROWS):
                xt = xtmp_pool.tile([C, QROWS, W], fp32, name="xt")
                nc.sync.dma_start(
                    out=xt, in_=x_d[b, :, q * QROWS : (q + 1) * QROWS, :]
                )
                nc.vector.tensor_copy(
                    x_pad[:, 1 + q * QROWS : 1 + (q + 1) * QROWS, 1 : W + 1], xt
                )

            # ---- pooled mean (halo zeros don't change the sum) ----
            psum_col = smalls.tile([C, 1], fp32, name=f"psum_col{b}")
            nc.vector.reduce_sum(out=psum_col, in_=x_pad.rearrange("c h w -> c (h w)"), axis=AX.X)
            pooled = smalls.tile([C, 1], fp32, name=f"pooled{b}")
            nc.scalar.mul(pooled, psum_col, 1.0 / (H * W))

            # ---- attention MLP ----
            h_ps = spsum.tile([HID, 1], fp32, name=f"h_ps{b}", tag="sps")
            nc.tensor.matmul(h_ps, w1T, pooled, start=True, stop=True)
            h_sb = smalls.tile([HID, 1], fp32, name=f"h_sb{b}")
            nc.scalar.activation(h_sb, h_ps, AF.Relu)

            log_ps = spsum.tile([1, K], fp32, name=f"log_ps{b}", tag="sps")
            nc.tensor.matmul(log_ps, h_sb, w2T, start=True, stop=True)

            # softmax over free dim (K=4), temperature 30
            lmax = smalls.tile([1, 1], fp32, name=f"lmax{b}")
            nc.vector.reduce_max(out=lmax, in_=log_ps, axis=AX.X)
            negb = smalls.tile([1, 1], fp32, name=f"negb{b}")
            nc.scalar.mul(negb, lmax, -1.0 / TEMP)
            att_e = smalls.tile([1, K], fp32, name=f"att_e{b}")
            nc.scalar.activation(att_e, log_ps, AF.Exp, bias=negb, scale=1.0 / TEMP)
            esum = smalls.tile([1, 1], fp32, name=f"esum{b}")
            nc.vector.reduce_sum(out=esum, in_=att_e, axis=AX.X)
            rsum = smalls.tile([1, 1], fp32, name=f"rsum{b}")
            nc.vector.reciprocal(rsum, esum)
            att_row = smalls.tile([1, K], fp32, name=f"att_row{b}")
            nc.vector.tensor_scalar_mul(att_row, att_e, rsum)

            # broadcast att row to all 128 partitions via K=1 matmul
            attb_ps = spsum.tile([C, K], fp32, name=f"attb_ps{b}", tag="sps")
            nc.tensor.matmul(attb_ps, ones_row, att_row, start=True, stop=True)
            att_bc = smalls.tile([C, K], fp32, name=f"att_bc{b}")
            nc.vector.tensor_copy(att_bc, attb_ps)

            # aggregated bias [C, 1]
            btmp = smalls.tile([C, K], fp32, name=f"btmp{b}")
            nc.vector.tensor_mul(btmp, biasT, att_bc)
            aggb = smalls.tile([C, 1], fp32, name=f"aggb{b}")
            nc.vector.reduce_sum(out=aggb, in_=btmp, axis=AX.X)

            # ---- expert mixing fused with transpose ----
            # scaled identities att_k * I
            sids = []
            for k in range(K):
                sid = wpool.tile([C, C], fp32, name=f"sid{b}_{k}")
                nc.vector.tensor_scalar_mul(sid, ident, att_bc[:, k : k + 1])
                sids.append(sid)

            wT = []
            for s in range(TAPS):
                agg_ps = apsum.tile([C, C], fp32, name="agg_ps")
                for k in range(K):
                    nc.tensor.matmul(
                        agg_ps,
                        w_sb[k][:, :, s],
                        sids[k],
                        start=(k == 0),
                        stop=(k == K - 1),
                    )
                wt = wpool.tile([C, C], conv_dt, name=f"wT{b}_{s}")
                nc.vector.tensor_copy(wt, agg_ps)
                wT.append(wt)

            # ---- convolution ----
            for chunk in range(NCHUNK):
                h0 = chunk * ROWS_PER_CHUNK
                cps = cpsum.tile([C, ROWS_PER_CHUNK * W], fp32, name="cps")
                for s in range(TAPS):
                    dy, dx = s // KS, s % KS
                    rhs = x_pad[:, h0 + dy : h0 + dy + ROWS_PER_CHUNK, dx : dx + W]
                    nc.tensor.matmul(
                        cps,
                        wT[s],
                        rhs,
                        start=(s == 0),
                        stop=(s == TAPS - 1),
                    )
                og = stage.tile([C, ROWS_PER_CHUNK * W], fp32, name="og")
                nc.vector.tensor_scalar_add(og, cps, aggb)
                nc.sync.dma_start(
                    out=out_flat[b, :, h0 * W : (h0 + ROWS_PER_CHUNK) * W], in_=og
                )

    nc.compile()
    return nc


def _get_prog():
    conv_f32r = os.environ.get("KERNEL_CONV_DTYPE", "f32r") == "f32r"
    key = conv_f32r
    if key not in _cache:
        _cache[key] = _build(conv_f32r)
    return _cache[key]


def kernel(x, att_w1, att_w2, weight, bias):
    from concourse.bass_utils import run_bass_kernel_spmd

    nc = _get_prog()
    in_maps = []
    for i in range(N_CORES):
        in_maps.append(
            {
                "x": np.ascontiguousarray(x[i * BPC : (i + 1) * BPC]),
                "att_w1": np.asarray(att_w1),
                "att_w2": np.asarray(att_w2),
                "weight": np.asarray(weight),
                "bias": np.asarray(bias),
            }
        )
    res = run_bass_kernel_spmd(nc, in_maps, list(range(N_CORES)))
    kernel.last_results = res
    return np.concatenate([r["out"] for r in res.results], axis=0)


# revision 13
# speedup vs baseline: 84.2654x; 84.2654x over previous
# Dynamic convolution (CondConv-style) Trainium2 Bass kernel.
#
# Problem: x [16, 128, 128, 128]; per-sample attention over K=4 expert
# 3x3 conv kernels; per-sample aggregated conv + bias.
#
# Strategy: data-parallel over batch, 2 samples per core on 8 cores.
# Per core / per sample:
#   1. DMA x into SBUF as [C=128, 130, 130] with a zero halo (pad=1).
#   2. DVE reduce_sum over free dims -> pooled mean [C, 1].
#   3. Attention: two tiny matmuls + softmax over K=4 (free dim).
#   4. Expert mixing fused with transpose: for each of 9 taps,
#      agg_wT[ci, co] = sum_k att_k * W_k[co, ci, tap] via 4 accumulating
#      PE transpose-matmuls whose moving operand is att_k * I.
#   5. Conv: for each 4-row output chunk (512 cols), 9 PSUM-accumulated
#      matmuls rhs = shifted window of the padded x.
#   6. Drain: bias add (per-partition scalar) PSUM->SBUF, DMA to DRAM.
import os

import numpy as np

B, C, H, W = 16, 128, 128, 128
K, HID, KS = 4, 64, 3
TEMP = 30.0
N_CORES = 8
BPC = B // N_CORES  # samples per core
HP, WP = H + 2, W + 2  # padded spatial
ROWS_PER_CHUNK = 4
NCHUNK = H // ROWS_PER_CHUNK
TAPS = KS * KS

_cache = {}


def _build(conv_f32r: bool, repeat: int = 1):
    """Build + compile the Bass program (same program for all 8 cores)."""
    import concourse.bacc as bacc
    import concourse.mybir as mybir
    import concourse.tile as tile
    from concourse.masks import make_identity

    fp32 = mybir.dt.float32
    f32r = mybir.dt.float32r
    AF = mybir.ActivationFunctionType
    AX = mybir.AxisListType

    nc = bacc.Bacc(
        "TRN2",
        target_bir_lowering=False,
        debug=False,
        enable_asserts=False,
        num_devices=N_CORES,
    )

    x_d = nc.dram_tensor("x", (BPC, C, H, W), fp32, kind="ExternalInput").ap()
    w1_d = nc.dram_tensor("att_w1", (HID, C), fp32, kind="ExternalInput").ap()
    w2_d = nc.dram_tensor("att_w2", (K, HID), fp32, kind="ExternalInput").ap()
    wgt_d = nc.dram_tensor("weight", (K, C, C, KS, KS), fp32, kind="ExternalInput").ap()
    bias_d = nc.dram_tensor("bias", (K, C), fp32, kind="ExternalInput").ap()
    out_d = nc.dram_tensor("out", (BPC, C, H, W), fp32, kind="ExternalOutput").ap()

    wgt_flat = wgt_d.rearrange("k o i kh kw -> k o (i kh kw)")
    out_flat = out_d.rearrange("b c h w -> b c (h w)")

    # fp32r matmul operands must be written by a compute op that rounds to
    # fp32r; DMA alone does not qualify. So x goes HBM->SBUF contiguous,
    # then a DVE copy re-lays it into the padded tile with fp32r output.
    conv_dt = f32r if conv_f32r else fp32

    from contextlib import ExitStack

    with tile.TileContext(nc) as tc, ExitStack() as ctx:
        consts = ctx.enter_context(tc.tile_pool(name="consts", bufs=1))
        xpool = ctx.enter_context(tc.tile_pool(name="xpool", bufs=1))
        wpool = ctx.enter_context(tc.tile_pool(name="wpool", bufs=1))
        smalls = ctx.enter_context(tc.tile_pool(name="smalls", bufs=1))
        stage = ctx.enter_context(tc.tile_pool(name="stage", bufs=6))
        cpsum = ctx.enter_context(tc.tile_pool(name="cpsum", bufs=5, space="PSUM"))
        apsum = ctx.enter_context(tc.tile_pool(name="apsum", bufs=2, space="PSUM"))
        spsum = ctx.enter_context(tc.tile_pool(name="spsum", bufs=1, space="PSUM"))

        # ---- global constants ----
        ident = consts.tile([C, C], fp32, name="ident")
        make_identity(nc, ident)
        ones_row = consts.tile([1, C], fp32, name="ones_row")
        nc.vector.memset(ones_row, 1.0)
        zero_col = consts.tile([C, HP], fp32, name="zero_col")
        nc.vector.memset(zero_col, 0.0)

        w1T = consts.tile([C, HID], fp32, name="w1T")
        nc.sync.dma_start(out=w1T, in_=w1_d.rearrange("h c -> c h"))
        w2T = consts.tile([HID, K], fp32, name="w2T")
        nc.sync.dma_start(out=w2T, in_=w2_d.rearrange("k h -> h k"))
        bias_sb = consts.tile([K, C], fp32, name="bias_sb")
        nc.sync.dma_start(out=bias_sb, in_=bias_d)

        # bias transposed to [C(out), K] via PE transpose
        biasT_ps = spsum.tile([C, K], fp32, name="biasT_ps", tag="sps")
        nc.tensor.matmul(
            biasT_ps, bias_sb, ident[:K, :K], is_transpose=True, start=True, stop=True
        )
        biasT = consts.tile([C, K], fp32, name="biasT")
        nc.vector.tensor_copy(biasT, biasT_ps)

        # expert weight bank, [co, ci, tap] per expert (contiguous DMA)
        w_sb = []
        for k in range(K):
            wk = wpool.tile([C, C, TAPS], fp32, name=f"w_sb{k}")
            nc.sync.dma_start(out=wk, in_=wgt_flat[k].rearrange("o (i t) -> o i t", t=TAPS))
            w_sb.append(wk)

        xtmp_pool = ctx.enter_context(tc.tile_pool(name="xtmp", bufs=3))
        QROWS = 16  # x staging chunk height
        for b in [b for _ in range(repeat) for b in range(BPC)]:
            # ---- load x (contiguous DMA), re-lay into padded tile ----
            x_pad = xpool.tile([C, HP, WP], conv_dt, name=f"x_pad{b}")
            nc.vector.tensor_copy(x_pad[:, 0, :], zero_col)
            nc.vector.tensor_copy(x_pad[:, HP - 1, :], zero_col)
            nc.vector.tensor_copy(x_pad[:, :, 0], zero_col)
            nc.vector.tensor_copy(x_pad[:, :, WP - 1], zero_col)
            for q in range(H // QROWS):
                xt = xtmp_pool.tile([C, QROWS, W], fp32, name="xt")
                nc.sync.dma_start(
                    out=xt, in_=x_d[b, :, q * QROWS : (q + 1) * QROWS, :]
                )
                nc.vector.tensor_copy(
                    x_pad[:, 1 + q * QROWS : 1 + (q + 1) * QROWS, 1 : W + 1], xt
                )

            # ---- pooled mean (halo zeros don't change the sum) ----
            psum_col = smalls.tile([C, 1], fp32, name=f"psum_col{b}")
            nc.vector.reduce_sum(out=psum_col, in_=x_pad.rearrange("c h w -> c (h w)"), axis=AX.X)
            pooled = smalls.tile([C, 1], fp32, name=f"pooled{b}")
            nc.scalar.mul(pooled, psum_col, 1.0 / (H * W))

            # ---- attention MLP ----
            h_ps = spsum.tile([HID, 1], fp32, name=f"h_ps{b}", tag="sps")
            nc.tensor.matmul(h_ps, w1T, pooled, start=True, stop=True)
            h_sb = smalls.tile([HID, 1], fp32, name=f"h_sb{b}")
            nc.scalar.activation(h_sb, h_ps, AF.Relu)

            log_ps = spsum.tile([1, K], fp32, name=f"log_ps{b}", tag="sps")
            nc.tensor.matmul(log_ps, h_sb, w2T, start=True, stop=True)

            # softmax over free dim (K=4), temperature 30
            lmax = smalls.tile([1, 1], fp32, name=f"lmax{b}")
            nc.vector.reduce_max(out=lmax, in_=log_ps, axis=AX.X)
            negb = smalls.tile([1, 1], fp32, name=f"negb{b}")
            nc.scalar.mul(negb, lmax, -1.0 / TEMP)
            att_e = smalls.tile([1, K], fp32, name=f"att_e{b}")
            nc.scalar.activation(att_e, log_ps, AF.Exp, bias=negb, scale=1.0 / TEMP)
            esum = smalls.tile([1, 1], fp32, name=f"esum{b}")
            nc.vector.reduce_sum(out=esum, in_=att_e, axis=AX.X)
            rsum = smalls.tile([1, 1], fp32, name=f"rsum{b}")
            nc.vector.reciprocal(rsum, esum)
            att_row = smalls.tile([1, K], fp32, name=f"att_row{b}")
            nc.vector.tensor_scalar_mul(att_row, att_e, rsum)

            # broadcast att row to all 128 partitions via K=1 matmul
            attb_ps = spsum.tile([C, K], fp32, name=f"attb_ps{b}", tag="sps")
            nc.tensor.matmul(attb_ps, ones_row, att_row, start=True, stop=True)
            att_bc = smalls.tile([C, K], fp32, name=f"att_bc{b}")
            nc.vector.tensor_copy(att_bc, attb_ps)

            # aggregated bias [C, 1]
            btmp = smalls.tile([C, K], fp32, name=f"btmp{b}")
            nc.vector.tensor_mul(btmp, biasT, att_bc)
            aggb = smalls.tile([C, 1], fp32, name=f"aggb{b}")
            nc.vector.reduce_sum(out=aggb, in_=btmp, axis=AX.X)

            # ---- expert mixing fused with transpose ----
            # scaled identities att_k * I
            sids = []
            for k in range(K):
                sid = wpool.tile([C, C], fp32, name=f"sid{b}_{k}")
                nc.vector.tensor_scalar_mul(sid, ident, att_bc[:, k : k + 1])
                sids.append(sid)

            wT = []
            for s in range(TAPS):
                agg_ps = apsum.tile([C, C], fp32, name="agg_ps")
                for k in range(K):
                    nc.tensor.matmul(
                        agg_ps,
                        w_sb[k][:, :, s],
                        sids[k],
                        start=(k == 0),
                        stop=(k == K - 1),
                    )
                wt = wpool.tile([C, C], conv_dt, name=f"wT{b}_{s}")
                nc.vector.tensor_copy(wt, agg_ps)
                wT.append(wt)

            # ---- convolution ----
            for chunk in range(NCHUNK):
                h0 = chunk * ROWS_PER_CHUNK
                cps = cpsum.tile([C, ROWS_PER_CHUNK * W], fp32, name="cps")
                for s in range(TAPS):
                    dy, dx = s // KS, s % KS
                    rhs = x_pad[:, h0 + dy : h0 + dy + ROWS_PER_CHUNK, dx : dx + W]
                    nc.tensor.matmul(
                        cps,
                        wT[s],
                        rhs,
                        start=(s == 0),
                        stop=(s == TAPS - 1),
                    )
                og = stage.tile([C, ROWS_PER_CHUNK * W], fp32, name="og")
                nc.vector.tensor_scalar_add(og, cps, aggb)
                nc.sync.dma_start(
                    out=out_flat[b, :, h0 * W : (h0 + ROWS_PER_CHUNK) * W], in_=og
                )

    nc.compile()
    return nc


def _get_prog():
    conv_f32r = os.environ.get("KERNEL_CONV_DTYPE", "f32r") == "f32r"
    repeat = int(os.environ.get("KERNEL_REPEAT", "1"))
    key = (conv_f32r, repeat)
    if key not in _cache:
        _cache[key] = _build(conv_f32r, repeat)
    return _cache[key]


def kernel(x, att_w1, att_w2, weight, bias):
    from concourse.bass_utils import run_bass_kernel_spmd

    nc = _get_prog()
    in_maps = []
    for i in range(N_CORES):
        in_maps.append(
            {
                "x": np.ascontiguousarray(x[i * BPC : (i + 1) * BPC]),
                "att_w1": np.asarray(att_w1),
                "att_w2": np.asarray(att_w2),
                "weight": np.asarray(weight),
                "bias": np.asarray(bias),
            }
        )
    res = run_bass_kernel_spmd(nc, in_maps, list(range(N_CORES)))
    kernel.last_results = res
    return np.concatenate([r["out"] for r in res.results], axis=0)
